# revision 30
# baseline (speedup 1.0000x reference)
"""TRN2 Bass kernel for nn_Attention_86260123173325.

Single-head attention over N=4096 tokens, feature dim HW=4096:
  q, k, v = x[:,0], x[:,1], x[:,2] reshaped to [4096, 4096]
  out = softmax(0.5 * q @ k.T) @ v

Sharding: q rows split across 8 cores (512 rows each); k, v replicated.
Host-side marshaling pre-transposes q and k into PE-ready contraction-major
layouts (the PE reduces along the partition dim), and converts v to bf16
(phase-2 value quantization contributes <1e-3 output error while halving the
v HBM stream, which paces phase 2 otherwise).

Per-core algorithm (phase-1 matmuls in f32r = TF32-like; phase 2 in bf16):
  - Phase 1, per 128-row k block j: R^T[j,:] = k_j @ q^T via 32 accumulated
    f32r matmuls. Keep R^T in SBUF (fp32), and accumulate a row statistic
    W_i = sum_j exp(0.1*R_ij - 40) (exp on ACT, summed on gpsimd, one final
    ones^T matmul).  The -40 bias keeps W far below ~2^64 where the HW
    exp/f32r/ln chain was observed to break.  kT blocks stream as two 1MB
    DMAs on the two HWDGE rings (sync+scalar, bursts ~400GB/s); the first
    block and the 16 qT half-chunks are interleaved need-order so the PE
    starts ~10us in and stays fed through the ~10.4MB startup transient.
  - shift_i = 5*(ln(W_i) + 40) >= rowmax_i; any per-row shift cancels in the
    final normalization, so exp(dp - shift) is an exact softmax numerator.
  - Bridge: throwaway matmuls (pinned on ett30 so they cannot be hoisted)
    keep the PE busy through the wacc/ln serial chain so the HAM clock gate
    never re-throttles (a >3.4us PE idle gap costs ~35us of half-clock).
  - Pass 2 (bf16): eT = exp(0.5*R - 10*lnW - 400), shift-sub fused into one
    DVE scalar_tensor_tensor + exp bias; blocks 0/1 in 256-wide halves so
    the first phase-2 matmul trails the ln by ~1.5us.
  - Phase 2: O = (E @ v) * (1/rowsum) in passes sized to the 8 PSUM banks:
      A: rowsum(ib0,ib1) + O[ib0/1, cols 0:1024]   (races the exp pass)
      B: rowsum(ib2,ib3) + O[ib2/3, cols 0:1024]   (v tiles reused from A)
      then six passes O[all ib, one 512-col block each] for cols 1024:4096
      (4 of 6 shared PSUM banks active, 2 spare so passes overlap).
    Rowsum matmuls ride the same weight loads in dedicated banks (sharing
    a bank across passes is a fatal PE-write/DVE-read conflict); rinv is
    per-ib-pair so pass-A banks release before pass B needs them; the
    scaled psum->sbuf output copies alternate DVE / ACT(Copy, scale=rinv)
    to halve release latency at pass boundaries.
"""
import sys

sys.path.insert(0, "/opt/trn_rl_repo")

import ml_dtypes
import numpy as np

import concourse.tile as tile
from concourse import bacc, mybir
from concourse.bass_utils import run_bass_kernel_spmd

F32 = mybir.dt.float32
F32R = mybir.dt.float32r
BF16 = mybir.dt.bfloat16
EXP = mybir.ActivationFunctionType.Exp
LN = mybir.ActivationFunctionType.Ln

N_CORES = 8
N = 4096          # tokens (keys)
D = 4096          # feature dim (H*W)
M = N // N_CORES  # q rows per core = 512
NJ = N // 128     # 32 key blocks
ND = D // 128     # 32 feature blocks
NI = M // 128     # 4 q-row blocks per core
T_STAT = 0.2      # stage-1 temperature: exp(t*dp - 40) = exp(0.1*R - 40)
STAT_BIAS = 40.0
N_BRIDGE = 10     # PE keep-warm matmuls across the softmax serial chain


def _build_nc():
    nc = bacc.Bacc(None, target_bir_lowering=False, debug=False)

    # qT[p, db, i] = q[i, db*128+p]; kT[jb, p, db, jj] = k[jb*128+jj, db*128+p]
    qT_dram = nc.dram_tensor("qT", [128, ND, M], F32R, kind="ExternalInput")
    kT_dram = nc.dram_tensor("kT", [NJ, 128, ND, 128], F32R, kind="ExternalInput")
    v_dram = nc.dram_tensor("v", [N, D], BF16, kind="ExternalInput")
    o_dram = nc.dram_tensor("o", [M, D], F32, kind="ExternalOutput")

    with tile.TileContext(nc) as tc:
        with tc.tile_pool(name="persist", bufs=1) as persist:
            # R^T storage, [j-within-block, j-block, i] (fp32, exact scores)
            s_sb = persist.tile([128, NJ, M], F32)

            ones_f = persist.tile([128, 128], F32, tag="ones_f")
            nc.vector.memset(ones_f[:], 1.0)
            # all-ones f32r [128,128]: W-stat lhsT (output lands broadcast on
            # all 128 partitions) + bridge matmuls
            ones_r = persist.tile([128, 128], F32R, tag="ones_r")
            nc.vector.tensor_copy(ones_r[:], ones_f[:])
            # bf16 ones [128,2]: phase-2 rowsum rhs
            ones_h = persist.tile([128, 2], BF16, tag="ones_h")
            nc.vector.tensor_copy(ones_h[:], ones_f[:, 0:2])

            zero_b = persist.tile([128, 1], F32, tag="zero_b")
            nc.vector.memset(zero_b[:], 0.0)

            # stage-1 exp bias: keeps W = sum exp(0.2*dp - 40) well under
            # ~2^64, where the HW exp/f32r-matmul/ln chain breaks
            stat_b = persist.tile([128, 1], F32, tag="stat_b")
            nc.vector.memset(stat_b[:], -STAT_BIAS)

            # exp bias for pass 2: exp(0.5*(R - 10*lnW - 400))
            #                    = exp(0.5*R - 5*lnW - 200)
            e2_b = persist.tile([128, 1], F32, tag="e2_b")
            nc.vector.memset(e2_b[:], -0.5 * STAT_BIAS * 2.0 / T_STAT)
            w_ln = persist.tile([128, M], F32, tag="w_ln")
            rsum = persist.tile([128, NI], F32, tag="rsum")
            rinv = persist.tile([128, NI], F32, tag="rinv")
            br_sb = persist.tile([128, 2], F32, tag="br_sb")

            # ---------------- phase 1: R^T blocks + W stats ----------------
            with (
                tc.tile_pool(name="qT", bufs=1) as qTpool,
                tc.tile_pool(name="kT", bufs=4) as kTpool,
                tc.tile_pool(name="ett", bufs=2) as etpool,
                tc.tile_pool(name="psS", bufs=2, space="PSUM") as psS,
                tc.tile_pool(name="psW", bufs=1, space="PSUM") as psWp,
                tc.tile_pool(name="psBr", bufs=1, space="PSUM") as psBr,
            ):
                # qT in 16 half-chunk tiles (0.5MB each) -> fine-grained
                # deps.  DMA emission is need-order interleaved with the
                # first kT blocks across both HWDGE rings so block 0's
                # matmul dblk=2b starts about when chunk b lands.
                NQ = 16
                qT_parts = [
                    qTpool.tile([128, ND // NQ, M], F32R, tag=f"qT{b}",
                                name=f"qT{b}")
                    for b in range(NQ)
                ]

                def qT_dma(b, eng):
                    eng.dma_start(
                        out=qT_parts[b][:],
                        in_=qT_dram[:, b * (ND // NQ):(b + 1) * (ND // NQ), :],
                    )

                def qT_slice(dblk):
                    return qT_parts[dblk // (ND // NQ)][:, dblk % (ND // NQ), :]

                kT_tiles = {}

                def kT_dma(j, part, eng, nsl=2):
                    if j not in kT_tiles:
                        kT_tiles[j] = kTpool.tile([128, ND, 128], F32R,
                                                  tag="kT", name=f"kT{j}")
                    kt = kT_tiles[j]
                    step = ND // nsl
                    eng.dma_start(
                        out=kt[:, part * step:(part + 1) * step, :],
                        in_=kT_dram[j][:, part * step:(part + 1) * step, :],
                    )

                # first pieces extra small: first matmul needs only kT0's
                # first 4 dblks (0.25MB) + qT chunk 0 (0.5MB)
                kT_dma(0, 0, nc.scalar, nsl=8)   # dblk 0:4
                qT_dma(0, nc.scalar)
                kT_dma(0, 1, nc.sync, nsl=8)     # dblk 4:8
                qT_dma(1, nc.sync)
                kT_dma(0, 1, nc.scalar, nsl=4)   # dblk 8:16
                qT_dma(2, nc.scalar)
                kT_dma(0, 2, nc.sync, nsl=4)     # dblk 16:24
                qT_dma(3, nc.sync)
                kT_dma(0, 3, nc.scalar, nsl=4)   # dblk 24:32
                for b in range(4, NQ):
                    qT_dma(b, nc.scalar if b % 2 == 0 else nc.sync)
                    if b == 10:
                        kT_dma(1, 0, nc.sync)
                    elif b == 11:
                        kT_dma(1, 1, nc.scalar)

                psW = psWp.tile([128, M], F32)
                # wacc[jj, i] = sum_{j-blocks} exp(0.1*R - 40); the
                # cross-partition reduction to W happens in one matmul
                wacc = persist.tile([128, M], F32, tag="wacc")
                ett31 = None
                for j in range(NJ):
                    if j > 0 and j not in kT_tiles:
                        kT_dma(j, 0, nc.sync)
                        kT_dma(j, 1, nc.scalar)
                    kT = kT_tiles[j]
                    ps = psS.tile([128, M], F32, tag="S", name=f"ps{j}")
                    for dblk in range(ND):
                        nc.tensor.matmul(
                            ps[:],
                            kT[:, dblk, :],
                            qT_slice(dblk),
                            start=(dblk == 0),
                            stop=(dblk == ND - 1),
                        )
                    # stash raw scores R^T (fp32)
                    nc.vector.tensor_copy(s_sb[:, j, :], ps[:])
                    # W stat: exp(0.1*R - 40), accumulated on the DVE
                    ett = etpool.tile([128, M], F32R, tag="ett", name=f"et{j}")
                    nc.scalar.activation(
                        out=ett[:], in_=ps[:], func=EXP,
                        bias=stat_b[:], scale=0.5 * T_STAT,
                    )
                    if j == 0:
                        nc.gpsimd.tensor_copy(wacc[:], ett[:])
                    else:
                        nc.gpsimd.tensor_add(wacc[:], wacc[:], ett[:])
                    if j == NJ - 2:
                        ett_pin = ett

                # PE bridge across the wacc/ln/shift serial chain: gated on
                # ett30 so it starts the moment the last score matmul ends,
                # overwritten in place so it serializes back-to-back on the
                # PE.  Split around the psW reduction (which waits on the
                # gpsimd wacc chain tail) so no PE-idle window exceeds the
                # ~3.4us HAM re-throttle threshold.
                br = psBr.tile([128, M], F32)
                for r in range(N_BRIDGE):
                    nc.tensor.matmul(
                        br[:], ones_r[:], ett_pin[:],
                        start=True, stop=True, skip_group_check=True,
                    )

                # cross-partition reduce: psW[p, i] = W_i (broadcast), f32
                nc.tensor.matmul(psW[:], ones_f[:], wacc[:],
                                 start=True, stop=True)

                for r in range(9):
                    nc.tensor.matmul(
                        br[:], ones_r[:], ett_pin[:],
                        start=True, stop=True, skip_group_check=True,
                    )
                nc.vector.tensor_copy(br_sb[:], br[:, 0:2])

                # lnW' (psW rows are identical so this lands broadcast);
                # the 10*lnW+400 shift is fused into the pass-2
                # scalar_tensor_tensor + exp bias.  Half tiles shorten the
                # serial chain ahead of the first phase-2 consumer.
                for h in range(2):
                    hs = slice(h * (M // 2), (h + 1) * (M // 2))
                    nc.scalar.activation(
                        out=w_ln[:, hs], in_=psW[:, hs], func=LN,
                        bias=zero_b[:], scale=1.0,
                    )

            # ---------------- phase 2: eT = exp(0.5*R - shift); O ----------
            with (
                tc.tile_pool(name="eTp", bufs=1) as eTpool,
                tc.tile_pool(name="tmp", bufs=4) as tmpool,
                tc.tile_pool(name="vq0", bufs=1) as vq0pool,
                tc.tile_pool(name="vrot", bufs=10) as vpool,
                tc.tile_pool(name="osb", bufs=8) as opool,
                tc.tile_pool(name="psO", bufs=6, space="PSUM") as psO,
                tc.tile_pool(name="psR", bufs=2, space="PSUM") as psR,
            ):
                eT_t = eTpool.tile([128, NJ, M], BF16, name="eT_t")

                def eT(j, i0, i1):
                    return eT_t[:, j, i0:i1]

                COPY = mybir.ActivationFunctionType.Copy

                def scale_store(ps_tile, ib, c0, label, on_act=False):
                    # alternating DVE / ACT halves the serial psum-release
                    # latency at pass boundaries
                    osb = opool.tile([128, 512], F32, tag="osb",
                                     name=f"ob_{label}")
                    if on_act:
                        nc.scalar.activation(
                            out=osb[:], in_=ps_tile[:], func=COPY,
                            bias=0.0, scale=rinv[:, ib:ib + 1],
                        )
                    else:
                        nc.vector.tensor_scalar_mul(
                            osb[:], ps_tile[:], rinv[:, ib:ib + 1]
                        )
                    nc.scalar.dma_start(
                        out=o_dram[ib * 128:(ib + 1) * 128, c0:c0 + 512],
                        in_=osb[:],
                    )

                def exp_block(j):
                    # tmp = s_sb - 10*lnW; exp applies *0.5 and -200 bias.
                    # First two blocks in halves so the first phase-2 matmul
                    # trails the ln by ~1.5us; the rest full-width (fewer
                    # DVE/ACT dispatches).
                    nh = 2 if j < 2 else 1
                    for h in range(nh):
                        hs = slice(h * (M // nh), (h + 1) * (M // nh))
                        tmp = tmpool.tile([128, M // nh], F32, tag="tmp",
                                          name=f"tmp{j}_{h}")
                        nc.vector.scalar_tensor_tensor(
                            tmp[:], w_ln[:, hs], -2.0 / T_STAT,
                            s_sb[:, j, hs],
                            mybir.AluOpType.mult, mybir.AluOpType.add,
                        )
                        nc.scalar.activation(
                            out=eT_t[:, j, hs], in_=tmp[:],
                            func=EXP, bias=e2_b[:], scale=0.5,
                        )

                # rowsums: per-(j, ib) tiny matmuls ride the pass A/B weight
                # loads.  Separate PSUM banks per pass — sharing one bank is
                # a fatal PE-write/DVE-read bank conflict (pass B's matmuls
                # race pass A's rowsum copies).

                # ---- pass A: exp pass + rowsum(ib0/1) + O[ib0/1, 0:1024]
                prA = [psR.tile([128, 2], F32, tag="pr", name=f"prA{ib}")
                       for ib in range(2)]
                oa = [psO.tile([128, 512], F32, tag="o", name=f"oa{t}")
                      for t in range(4)]
                vq_tiles = []
                for j in range(NJ):
                    vj = vq0pool.tile([128, 1024], BF16, tag=f"vq{j}",
                                      name=f"vq{j}")
                    nc.sync.dma_start(
                        out=vj[:], in_=v_dram[j * 128:(j + 1) * 128, 0:1024]
                    )
                    vq_tiles.append(vj)
                    exp_block(j)
                    for ib in range(2):
                        eTb = eT(j, ib * 128, (ib + 1) * 128)
                        nc.tensor.matmul(
                            prA[ib][:], eTb, ones_h[:],
                            start=(j == 0), stop=(j == NJ - 1),
                        )
                        for dq in range(2):
                            nc.tensor.matmul(
                                oa[ib * 2 + dq][:], eTb,
                                vj[:, dq * 512:(dq + 1) * 512],
                                start=(j == 0), stop=(j == NJ - 1),
                            )
                for ib in range(2):
                    nc.vector.tensor_copy(rsum[:, ib:ib + 1],
                                          prA[ib][:, 0:1])
                nc.vector.reciprocal(rinv[:, 0:2], rsum[:, 0:2])
                for ib in range(2):
                    for dq in range(2):
                        scale_store(oa[ib * 2 + dq], ib, dq * 512,
                                    f"A{ib}_{dq}", on_act=(dq == 1))

                # ---- pass B: rowsum(ib2/3) + O[ib2/3, 0:1024], v reused
                prB = [psR.tile([128, 2], F32, tag="pr", name=f"prB{ib}")
                       for ib in range(2)]
                ob = [psO.tile([128, 512], F32, tag="o", name=f"obt{t}")
                      for t in range(4)]
                for j in range(NJ):
                    vj = vq_tiles[j]
                    for ib in range(2, 4):
                        eTb = eT(j, ib * 128, (ib + 1) * 128)
                        nc.tensor.matmul(
                            prB[ib - 2][:], eTb, ones_h[:],
                            start=(j == 0), stop=(j == NJ - 1),
                        )
                        for dq in range(2):
                            nc.tensor.matmul(
                                ob[(ib - 2) * 2 + dq][:], eTb,
                                vj[:, dq * 512:(dq + 1) * 512],
                                start=(j == 0), stop=(j == NJ - 1),
                            )
                for ib in range(2, 4):
                    nc.vector.tensor_copy(rsum[:, ib:ib + 1],
                                          prB[ib - 2][:, 0:1])
                nc.vector.reciprocal(rinv[:, 2:4], rsum[:, 2:4])
                for ib in range(2, 4):
                    for dq in range(2):
                        scale_store(ob[(ib - 2) * 2 + dq], ib, dq * 512,
                                    f"B{ib}_{dq}", on_act=(dq == 1))

                # ---- six passes: O[all ib, one 512-col block], 1024:4096
                for p in range(6):
                    c0 = 1024 + p * 512
                    oc = [psO.tile([128, 512], F32, tag="o",
                                   name=f"oc{p}_{t}") for t in range(NI)]
                    for j in range(NJ):
                        vj = vpool.tile([128, 512], BF16, tag="v",
                                        name=f"v{p}_{j}")
                        nc.sync.dma_start(
                            out=vj[:],
                            in_=v_dram[j * 128:(j + 1) * 128, c0:c0 + 512],
                        )
                        for ib in range(NI):
                            nc.tensor.matmul(
                                oc[ib][:], eT(j, ib * 128, (ib + 1) * 128),
                                vj[:],
                                start=(j == 0), stop=(j == NJ - 1),
                            )
                    for ib in range(NI):
                        scale_store(oc[ib], ib, c0, f"C{p}_{ib}",
                                    on_act=(ib % 2 == 1))

    nc.compile()
    return nc


_NC_CACHE = None


def _get_nc():
    global _NC_CACHE
    if _NC_CACHE is None:
        _NC_CACHE = _build_nc()
    return _NC_CACHE


def _make_in_maps(x: np.ndarray) -> list:
    x = np.asarray(x)
    n, c, h, w = x.shape
    assert (n, c, h * w) == (N, 3, D), f"unexpected shape {x.shape}"
    xr = np.ascontiguousarray(x.reshape(n, c, h * w).transpose(1, 0, 2))
    q_full, k, v = xr[0], xr[1], xr[2]
    # kT[jb, p, db, jj] = k[jb*128+jj, db*128+p] -- per-(jb) contiguous 2MB
    kT = np.ascontiguousarray(
        k.reshape(NJ, 128, ND, 128).transpose(0, 3, 2, 1)
    )
    v_bf16 = v.astype(ml_dtypes.bfloat16)
    in_maps = []
    for core in range(N_CORES):
        qc = q_full[core * M:(core + 1) * M]          # [M, D]
        # qT[p, db, i] = q[i, db*128+p]
        qTc = np.ascontiguousarray(
            qc.reshape(M, ND, 128).transpose(2, 1, 0)
        )
        in_maps.append({"qT": qTc, "kT": kT, "v": v_bf16})
    return in_maps


def kernel(x: np.ndarray) -> np.ndarray:
    nc = _get_nc()
    res = run_bass_kernel_spmd(nc, _make_in_maps(x), core_ids=list(range(N_CORES)))
    out = np.concatenate([r["o"] for r in res.results], axis=0)
    return out.astype(np.float32)


# revision 34
# speedup vs baseline: 1.1868x; 1.1868x over previous
"""TRN2 Bass kernel for nn_Attention_86260123173325.

Single-head attention over N=4096 tokens, feature dim HW=4096:
  q, k, v = x[:,0], x[:,1], x[:,2] reshaped to [4096, 4096]
  out = softmax(0.5 * q @ k.T) @ v

Sharding: q rows split across 8 cores (512 rows each); k, v replicated.
Host-side marshaling pre-transposes q and k into PE-ready contraction-major
layouts (the PE reduces along the partition dim), and converts v to bf16
(phase-2 value quantization contributes <1e-3 output error while halving the
v HBM stream, which paces phase 2 otherwise).

Per-core algorithm (phase-1 matmuls in f32r = TF32-like; phase 2 in bf16):
  - Phase 1, per 128-row k block j: R^T[j,:] = k_j @ q^T via 32 accumulated
    f32r matmuls. Keep R^T in SBUF (fp32), and accumulate a row statistic
    W_i = sum_j exp(0.1*R_ij - 40) (exp on ACT, summed on gpsimd, one final
    ones^T matmul).  The -40 bias keeps W far below ~2^64 where the HW
    exp/f32r/ln chain was observed to break.  kT blocks stream as two 1MB
    DMAs on the two HWDGE rings (sync+scalar, bursts ~400GB/s); the first
    block and the 16 qT half-chunks are interleaved need-order so the PE
    starts ~10us in and stays fed through the ~10.4MB startup transient.
  - shift_i = 5*(ln(W_i) + 40) >= rowmax_i; any per-row shift cancels in the
    final normalization, so exp(dp - shift) is an exact softmax numerator.
  - Bridge: throwaway matmuls (pinned on ett30 so they cannot be hoisted)
    keep the PE busy through the wacc/ln serial chain so the HAM clock gate
    never re-throttles (a >3.4us PE idle gap costs ~35us of half-clock).
  - Pass 2 (bf16): eT = exp(0.5*R - 10*lnW - 400), shift-sub fused into one
    DVE scalar_tensor_tensor + exp bias; blocks 0/1 in 256-wide halves so
    the first phase-2 matmul trails the ln by ~1.5us.
  - Phase 2: O = (E @ v) * (1/rowsum) in passes sized to the 8 PSUM banks:
      A: rowsum(ib0,ib1) + O[ib0/1, cols 0:1024]   (races the exp pass)
      B: rowsum(ib2,ib3) + O[ib2/3, cols 0:1024]   (v tiles reused from A)
      then six passes O[all ib, one 512-col block each] for cols 1024:4096
      (4 of 6 shared PSUM banks active, 2 spare so passes overlap).
    Rowsum matmuls ride the same weight loads in dedicated banks (sharing
    a bank across passes is a fatal PE-write/DVE-read conflict); rinv is
    per-ib-pair so pass-A banks release before pass B needs them; the
    scaled psum->sbuf output copies alternate DVE / ACT(Copy, scale=rinv)
    to halve release latency at pass boundaries.
"""
import sys

sys.path.insert(0, "/opt/trn_rl_repo")

import ml_dtypes
import numpy as np

import concourse.tile as tile
from concourse import bacc, mybir
from concourse.bass_utils import run_bass_kernel_spmd

F32 = mybir.dt.float32
F32R = mybir.dt.float32r
BF16 = mybir.dt.bfloat16
EXP = mybir.ActivationFunctionType.Exp
LN = mybir.ActivationFunctionType.Ln

N_CORES = 8
N = 4096          # tokens (keys)
D = 4096          # feature dim (H*W)
M = N // N_CORES  # q rows per core = 512
NJ = N // 128     # 32 key blocks
ND = D // 128     # 32 feature blocks
NI = M // 128     # 4 q-row blocks per core
T_STAT = 0.2      # stage-1 temperature: exp(t*dp - 40) = exp(0.1*R - 40)
STAT_BIAS = 40.0
N_BRIDGE = 10     # PE keep-warm matmuls across the softmax serial chain


def _build_nc():
    nc = bacc.Bacc(None, target_bir_lowering=False, debug=False)

    # qT[p, db, i] = q[i, db*128+p]; kT[jb, p, db, jj] = k[jb*128+jj, db*128+p]
    qT_dram = nc.dram_tensor("qT", [128, ND, M], F32R, kind="ExternalInput")
    kT_dram = nc.dram_tensor("kT", [NJ, 128, ND, 128], F32R, kind="ExternalInput")
    v_dram = nc.dram_tensor("v", [N, D], BF16, kind="ExternalInput")
    o_dram = nc.dram_tensor("o", [M, D], F32, kind="ExternalOutput")

    with tile.TileContext(nc) as tc:
        with tc.tile_pool(name="persist", bufs=1) as persist:
            # R^T storage, [j-within-block, j-block, i] (fp32, exact scores)
            s_sb = persist.tile([128, NJ, M], F32)

            ones_f = persist.tile([128, 128], F32, tag="ones_f")
            nc.vector.memset(ones_f[:], 1.0)
            # all-ones f32r [128,128]: W-stat lhsT (output lands broadcast on
            # all 128 partitions) + bridge matmuls
            ones_r = persist.tile([128, 128], F32R, tag="ones_r")
            nc.vector.tensor_copy(ones_r[:], ones_f[:])
            # bf16 ones [128,2]: phase-2 rowsum rhs
            ones_h = persist.tile([128, 2], BF16, tag="ones_h")
            nc.vector.tensor_copy(ones_h[:], ones_f[:, 0:2])

            zero_b = persist.tile([128, 1], F32, tag="zero_b")
            nc.vector.memset(zero_b[:], 0.0)

            # stage-1 exp bias: keeps W = sum exp(0.2*dp - 40) well under
            # ~2^64, where the HW exp/f32r-matmul/ln chain breaks
            stat_b = persist.tile([128, 1], F32, tag="stat_b")
            nc.vector.memset(stat_b[:], -STAT_BIAS)

            # exp bias for pass 2: exp(0.5*(R - 10*lnW - 400))
            #                    = exp(0.5*R - 5*lnW - 200)
            e2_b = persist.tile([128, 1], F32, tag="e2_b")
            nc.vector.memset(e2_b[:], -0.5 * STAT_BIAS * 2.0 / T_STAT)
            w_ln = persist.tile([128, M], F32, tag="w_ln")
            rsum = persist.tile([128, NI], F32, tag="rsum")
            rinv = persist.tile([128, NI], F32, tag="rinv")
            br_sb = persist.tile([128, 2], F32, tag="br_sb")

            # ---------------- phase 1: R^T blocks + W stats ----------------
            with (
                tc.tile_pool(name="qT", bufs=1) as qTpool,
                tc.tile_pool(name="kT", bufs=4) as kTpool,
                tc.tile_pool(name="ett", bufs=2) as etpool,
                tc.tile_pool(name="psS", bufs=2, space="PSUM") as psS,
                tc.tile_pool(name="psW", bufs=1, space="PSUM") as psWp,
                tc.tile_pool(name="psBr", bufs=1, space="PSUM") as psBr,
            ):
                # qT in 16 half-chunk tiles (0.5MB each) -> fine-grained
                # deps.  DMA emission is need-order interleaved with the
                # first kT blocks across both HWDGE rings so block 0's
                # matmul dblk=2b starts about when chunk b lands.
                NQ = 16
                qT_parts = [
                    qTpool.tile([128, ND // NQ, M], F32R, tag=f"qT{b}",
                                name=f"qT{b}")
                    for b in range(NQ)
                ]

                def qT_dma(b, eng):
                    eng.dma_start(
                        out=qT_parts[b][:],
                        in_=qT_dram[:, b * (ND // NQ):(b + 1) * (ND // NQ), :],
                    )

                def qT_slice(dblk):
                    return qT_parts[dblk // (ND // NQ)][:, dblk % (ND // NQ), :]

                kT_tiles = {}

                def kT_dma(j, part, eng, nsl=2):
                    if j not in kT_tiles:
                        kT_tiles[j] = kTpool.tile([128, ND, 128], F32R,
                                                  tag="kT", name=f"kT{j}")
                    kt = kT_tiles[j]
                    step = ND // nsl
                    eng.dma_start(
                        out=kt[:, part * step:(part + 1) * step, :],
                        in_=kT_dram[j][:, part * step:(part + 1) * step, :],
                    )

                def qT_dma_half(b, h, eng):
                    eng.dma_start(
                        out=qT_parts[b][:, h:h + 1, :],
                        in_=qT_dram[:, 2 * b + h:2 * b + h + 1, :],
                    )

                # first pieces extra small: the first matmul needs only
                # kT0's first 4 dblks (0.25MB) + qT dblk 0 (0.25MB)
                kT_dma(0, 0, nc.scalar, nsl=8)   # dblk 0:4
                qT_dma_half(0, 0, nc.scalar)
                kT_dma(0, 1, nc.sync, nsl=8)     # dblk 4:8
                qT_dma_half(1, 0, nc.sync)
                qT_dma_half(0, 1, nc.scalar)
                qT_dma_half(1, 1, nc.sync)
                kT_dma(0, 1, nc.scalar, nsl=4)   # dblk 8:16
                qT_dma(2, nc.scalar)
                kT_dma(0, 2, nc.sync, nsl=4)     # dblk 16:24
                qT_dma(3, nc.sync)
                kT_dma(0, 3, nc.scalar, nsl=4)   # dblk 24:32
                for b in range(4, NQ):
                    qT_dma(b, nc.scalar if b % 2 == 0 else nc.sync)
                    if b == 10:
                        kT_dma(1, 0, nc.sync)
                    elif b == 11:
                        kT_dma(1, 1, nc.scalar)

                psW = psWp.tile([128, M], F32)
                # wacc[jj, i] = sum_{j-blocks} exp(0.1*R - 40); the
                # cross-partition reduction to W happens in one matmul
                wacc = persist.tile([128, M], F32, tag="wacc")
                ett31 = None
                for j in range(NJ):
                    if j > 0 and j not in kT_tiles:
                        kT_dma(j, 0, nc.sync)
                        kT_dma(j, 1, nc.scalar)
                    kT = kT_tiles[j]
                    ps = psS.tile([128, M], F32, tag="S", name=f"ps{j}")
                    for dblk in range(ND):
                        nc.tensor.matmul(
                            ps[:],
                            kT[:, dblk, :],
                            qT_slice(dblk),
                            start=(dblk == 0),
                            stop=(dblk == ND - 1),
                        )
                    # stash raw scores R^T (fp32)
                    nc.vector.tensor_copy(s_sb[:, j, :], ps[:])
                    # W stat: exp(0.1*R - 40), accumulated on the DVE
                    ett = etpool.tile([128, M], F32R, tag="ett", name=f"et{j}")
                    nc.scalar.activation(
                        out=ett[:], in_=ps[:], func=EXP,
                        bias=stat_b[:], scale=0.5 * T_STAT,
                    )
                    if j == 0:
                        nc.gpsimd.tensor_copy(wacc[:], ett[:])
                    else:
                        nc.gpsimd.tensor_add(wacc[:], wacc[:], ett[:])
                    if j == NJ - 2:
                        ett_pin = ett

                # PE bridge across the wacc/ln/shift serial chain: gated on
                # ett30 so it starts the moment the last score matmul ends,
                # overwritten in place so it serializes back-to-back on the
                # PE.  Split around the psW reduction (which waits on the
                # gpsimd wacc chain tail) so no PE-idle window exceeds the
                # ~3.4us HAM re-throttle threshold.
                br = psBr.tile([128, M], F32)
                for r in range(N_BRIDGE):
                    nc.tensor.matmul(
                        br[:], ones_r[:], ett_pin[:],
                        start=True, stop=True, skip_group_check=True,
                    )

                # cross-partition reduce: psW[p, i] = W_i (broadcast), f32
                nc.tensor.matmul(psW[:], ones_f[:], wacc[:],
                                 start=True, stop=True)

                for r in range(12):
                    nc.tensor.matmul(
                        br[:], ones_r[:], ett_pin[:],
                        start=True, stop=True, skip_group_check=True,
                    )
                nc.vector.tensor_copy(br_sb[:], br[:, 0:2])

                # lnW' (psW rows are identical so this lands broadcast);
                # the 10*lnW+400 shift is fused into the pass-2
                # scalar_tensor_tensor + exp bias.  Half tiles shorten the
                # serial chain ahead of the first phase-2 consumer.
                for h in range(2):
                    hs = slice(h * (M // 2), (h + 1) * (M // 2))
                    nc.scalar.activation(
                        out=w_ln[:, hs], in_=psW[:, hs], func=LN,
                        bias=zero_b[:], scale=1.0,
                    )

            # ---------------- phase 2: eT = exp(0.5*R - shift); O ----------
            with (
                tc.tile_pool(name="eTp", bufs=1) as eTpool,
                tc.tile_pool(name="tmp", bufs=4) as tmpool,
                tc.tile_pool(name="vq0", bufs=1) as vq0pool,
                tc.tile_pool(name="vrot", bufs=10) as vpool,
                tc.tile_pool(name="osb", bufs=8) as opool,
                tc.tile_pool(name="psO", bufs=6, space="PSUM") as psO,
                tc.tile_pool(name="psR", bufs=2, space="PSUM") as psR,
            ):
                eT_t = eTpool.tile([128, NJ, M], BF16, name="eT_t")

                def eT(j, i0, i1):
                    return eT_t[:, j, i0:i1]

                COPY = mybir.ActivationFunctionType.Copy

                def scale_store(ps_tile, ib, c0, label, on_act=False):
                    # alternating DVE / ACT halves the serial psum-release
                    # latency at pass boundaries
                    osb = opool.tile([128, 512], F32, tag="osb",
                                     name=f"ob_{label}")
                    if on_act:
                        nc.scalar.activation(
                            out=osb[:], in_=ps_tile[:], func=COPY,
                            bias=0.0, scale=rinv[:, ib:ib + 1],
                        )
                    else:
                        nc.vector.tensor_scalar_mul(
                            osb[:], ps_tile[:], rinv[:, ib:ib + 1]
                        )
                    nc.scalar.dma_start(
                        out=o_dram[ib * 128:(ib + 1) * 128, c0:c0 + 512],
                        in_=osb[:],
                    )

                def exp_block(j):
                    # tmp = s_sb - 10*lnW; exp applies *0.5 and -200 bias.
                    # First two blocks in halves so the first phase-2 matmul
                    # trails the ln by ~1.5us; the rest full-width (fewer
                    # DVE/ACT dispatches).
                    nh = 2 if j < 2 else 1
                    for h in range(nh):
                        hs = slice(h * (M // nh), (h + 1) * (M // nh))
                        tmp = tmpool.tile([128, M // nh], F32, tag="tmp",
                                          name=f"tmp{j}_{h}")
                        nc.vector.scalar_tensor_tensor(
                            tmp[:], w_ln[:, hs], -2.0 / T_STAT,
                            s_sb[:, j, hs],
                            mybir.AluOpType.mult, mybir.AluOpType.add,
                        )
                        nc.scalar.activation(
                            out=eT_t[:, j, hs], in_=tmp[:],
                            func=EXP, bias=e2_b[:], scale=0.5,
                        )

                # rowsums: per-(j, ib) tiny matmuls ride the pass A/B weight
                # loads.  Separate PSUM banks per pass — sharing one bank is
                # a fatal PE-write/DVE-read bank conflict (pass B's matmuls
                # race pass A's rowsum copies).

                # ---- pass A: exp pass + rowsum(ib0/1) + O[ib0/1, 0:1024]
                prA = [psR.tile([128, 2], F32, tag="pr", name=f"prA{ib}")
                       for ib in range(2)]
                oa = [psO.tile([128, 512], F32, tag="o", name=f"oa{t}")
                      for t in range(4)]
                vq_tiles = []
                for j in range(NJ):
                    vj = vq0pool.tile([128, 1024], BF16, tag=f"vq{j}",
                                      name=f"vq{j}")
                    nc.sync.dma_start(
                        out=vj[:], in_=v_dram[j * 128:(j + 1) * 128, 0:1024]
                    )
                    vq_tiles.append(vj)
                    exp_block(j)
                    for ib in range(2):
                        eTb = eT(j, ib * 128, (ib + 1) * 128)
                        nc.tensor.matmul(
                            prA[ib][:], eTb, ones_h[:],
                            start=(j == 0), stop=(j == NJ - 1),
                        )
                        for dq in range(2):
                            nc.tensor.matmul(
                                oa[ib * 2 + dq][:], eTb,
                                vj[:, dq * 512:(dq + 1) * 512],
                                start=(j == 0), stop=(j == NJ - 1),
                            )
                for ib in range(2):
                    nc.vector.tensor_copy(rsum[:, ib:ib + 1],
                                          prA[ib][:, 0:1])
                nc.vector.reciprocal(rinv[:, 0:2], rsum[:, 0:2])
                for ib in range(2):
                    for dq in range(2):
                        scale_store(oa[ib * 2 + dq], ib, dq * 512,
                                    f"A{ib}_{dq}", on_act=(dq == 1))

                # ---- pass B: rowsum(ib2/3) + O[ib2/3, 0:1024], v reused
                prB = [psR.tile([128, 2], F32, tag="pr", name=f"prB{ib}")
                       for ib in range(2)]
                ob = [psO.tile([128, 512], F32, tag="o", name=f"obt{t}")
                      for t in range(4)]
                for j in range(NJ):
                    vj = vq_tiles[j]
                    for ib in range(2, 4):
                        eTb = eT(j, ib * 128, (ib + 1) * 128)
                        nc.tensor.matmul(
                            prB[ib - 2][:], eTb, ones_h[:],
                            start=(j == 0), stop=(j == NJ - 1),
                        )
                        for dq in range(2):
                            nc.tensor.matmul(
                                ob[(ib - 2) * 2 + dq][:], eTb,
                                vj[:, dq * 512:(dq + 1) * 512],
                                start=(j == 0), stop=(j == NJ - 1),
                            )
                for ib in range(2, 4):
                    nc.vector.tensor_copy(rsum[:, ib:ib + 1],
                                          prB[ib - 2][:, 0:1])
                nc.vector.reciprocal(rinv[:, 2:4], rsum[:, 2:4])
                for ib in range(2, 4):
                    for dq in range(2):
                        scale_store(ob[(ib - 2) * 2 + dq], ib, dq * 512,
                                    f"B{ib}_{dq}", on_act=(dq == 1))

                # ---- six passes: O[all ib, one 512-col block], 1024:4096
                for p in range(6):
                    c0 = 1024 + p * 512
                    oc = [psO.tile([128, 512], F32, tag="o",
                                   name=f"oc{p}_{t}") for t in range(NI)]
                    for j in range(NJ):
                        vj = vpool.tile([128, 512], BF16, tag="v",
                                        name=f"v{p}_{j}")
                        nc.sync.dma_start(
                            out=vj[:],
                            in_=v_dram[j * 128:(j + 1) * 128, c0:c0 + 512],
                        )
                        for ib in range(NI):
                            nc.tensor.matmul(
                                oc[ib][:], eT(j, ib * 128, (ib + 1) * 128),
                                vj[:],
                                start=(j == 0), stop=(j == NJ - 1),
                            )
                    for ib in range(NI):
                        scale_store(oc[ib], ib, c0, f"C{p}_{ib}",
                                    on_act=(ib % 2 == 1))

    nc.compile()
    return nc


_NC_CACHE = None


def _get_nc():
    global _NC_CACHE
    if _NC_CACHE is None:
        _NC_CACHE = _build_nc()
    return _NC_CACHE


def _make_in_maps(x: np.ndarray) -> list:
    x = np.asarray(x)
    n, c, h, w = x.shape
    assert (n, c, h * w) == (N, 3, D), f"unexpected shape {x.shape}"
    xr = np.ascontiguousarray(x.reshape(n, c, h * w).transpose(1, 0, 2))
    q_full, k, v = xr[0], xr[1], xr[2]
    # kT[jb, p, db, jj] = k[jb*128+jj, db*128+p] -- per-(jb) contiguous 2MB
    kT = np.ascontiguousarray(
        k.reshape(NJ, 128, ND, 128).transpose(0, 3, 2, 1)
    )
    v_bf16 = v.astype(ml_dtypes.bfloat16)
    in_maps = []
    for core in range(N_CORES):
        qc = q_full[core * M:(core + 1) * M]          # [M, D]
        # qT[p, db, i] = q[i, db*128+p]
        qTc = np.ascontiguousarray(
            qc.reshape(M, ND, 128).transpose(2, 1, 0)
        )
        in_maps.append({"qT": qTc, "kT": kT, "v": v_bf16})
    return in_maps


def kernel(x: np.ndarray) -> np.ndarray:
    nc = _get_nc()
    res = run_bass_kernel_spmd(nc, _make_in_maps(x), core_ids=list(range(N_CORES)))
    out = np.concatenate([r["o"] for r in res.results], axis=0)
    return out.astype(np.float32)
